# revision 1
# baseline (speedup 1.0000x reference)
"""Trainium2 Bass kernel for nn_DirectEncodingModel (gnn_message_passing).
Dependency-class pipelined gather design, data-parallel over 8 cores.

Key changes vs v1:
  - Slots of each layer sorted by (source-class, group): x-sourced slots
    gather from xt at t=0 (off the critical path); h_c-sourced slots gather
    right after h_c is written. Critical path only carries the small
    h-class gathers (split over 2 SWDGE queues each).
  - Tight 128-slot blocks (groups may split across blocks; PSUM has_written
    accumulation merges partial group sums). 8 PSUM tiles of 128 outputs
    per layer -> full-width ACT tanh.
  - accT holds only h rows [3072, BS]; x rows always gather from xt input.
"""

import numpy as np
import ml_dtypes

B = 16384
IN = 512
G, F, O = 64, 24, 16
GO_G, GO_F, GO_O = 4, 48, 16
N_CORES = 8
BS = B // N_CORES
HROWS = 3 * G * O  # 3072
NCHUNK = BS // 512

_cache = {}


def _cls_of(v):
    return 0 if v < IN else 1 + (v - IN) // (G * O)


class MM:
    __slots__ = ("blk", "tile", "c0", "c1", "start", "w_off", "w", "stop")

    def __init__(self, blk, tile, c0, c1, start, w_off, w):
        self.blk, self.tile, self.c0, self.c1 = blk, tile, c0, c1
        self.start, self.w_off, self.w = start, w_off, w
        self.stop = False


def build_plan(idx1, idx2, idx3, idxo, W1, W2, W3, Wo, b1, b2, b3, bo):
    """Returns dict with packed arrays + per-layer block/mm/gather structure.

    Layer entries: dict(nblk, idx[int16 nblk*128], segs=[(cls, b0, b1, nidx)],
    mms=[MM...], n_groups, gpt (groups per psum tile)).
    """
    wcols = []          # growing list of stationary columns [128, w]
    bias_cols = []      # [128] per (layer, ptile)
    layers = []

    def pack_layer(idx, W, b, n_g, fan, o):
        gpt = 128 // o  # groups per psum tile (8 hidden, 8 out? o=16 -> 8)
        slots = []
        for g in range(n_g):
            for f in range(fan):
                v = int(idx[g, f])
                slots.append((_cls_of(v), g, v, W[g, f, :]))
        slots.sort(key=lambda s: (s[0], s[1]))
        # segment by class, pad each to 128
        segs_raw = {}
        for s in slots:
            segs_raw.setdefault(s[0], []).append(s)
        padded = []
        segs = []
        for c in sorted(segs_raw):
            lst = segs_raw[c]
            b0 = len(padded) // 128
            while len(lst) % 128:
                lst.append((c, None, 0 if c == 0 else IN, np.zeros(o)))
            padded.extend(lst)
            b1 = len(padded) // 128
            segs.append((c, b0, b1, (b1 - b0) * 128))
        nblk = len(padded) // 128
        idx_arr = np.zeros(nblk * 128, np.int16)
        for i, s in enumerate(padded):
            v = s[2]
            idx_arr[i] = v if s[0] == 0 else v - IN
        # matmul segments per block: one mm per (block, psum tile touched),
        # col range 32-aligned. The first mm of each psum tile covers the
        # full used width with start=True (clears stale has_written);
        # everything else start=False (has_written-accumulate).
        mms = []
        used_w = min(gpt, n_g) * o  # used psum cols (128 hidden, 64 out)
        tile_first_pending = set(range((n_g + gpt - 1) // gpt))
        for blk in range(nblk):
            bslots = padded[blk * 128:(blk + 1) * 128]
            groups = sorted({s[1] for s in bslots if s[1] is not None})
            if not groups:
                continue
            tiles = sorted({g // gpt for g in groups})
            for t in tiles:
                tg = [g for g in groups if g // gpt == t]
                ga, gb = tg[0], tg[-1]
                first = t in tile_first_pending
                if first:
                    tile_first_pending.discard(t)
                    c0, c1 = 0, used_w
                else:
                    c0 = ((ga % gpt) * o) // 32 * 32
                    c1 = -((-((gb % gpt + 1) * o)) // 32) * 32
                    c1 = min(c1, used_w)
                # decompose into buddy-aligned [b0, b0+bw) pieces
                # (bw in {32,64,128}, b0 % bw == 0) per PE tile_position rules
                pieces = []
                p0 = c0
                while p0 < c1:
                    for bw in (128, 64, 32):
                        if p0 % bw == 0 and p0 + bw <= ((c1 + 31) // 32) * 32:
                            pieces.append((p0, min(p0 + bw, c1)))
                            p0 = p0 + bw
                            break
                for (p0c, p1c) in pieces:
                    w = p1c - p0c
                    stat = np.zeros((128, w), np.float32)
                    for r, s in enumerate(bslots):
                        if s[1] is not None and s[1] // gpt == t:
                            cc = (s[1] % gpt) * o - p0c
                            if 0 <= cc <= w - o:
                                stat[r, cc:cc + o] = s[3]
                    w_off = sum(wc.shape[1] for wc in wcols)
                    wcols.append(stat)
                    mms.append(MM(blk, t, p0c, p1c, first, w_off, w))
        last_per_tile = {}
        for i, m in enumerate(mms):
            last_per_tile[m.tile] = i
        for i in last_per_tile.values():
            mms[i].stop = True
        # bias per psum tile
        ntile = (n_g + gpt - 1) // gpt
        bt0 = len(bias_cols)
        for t in range(ntile):
            col = np.zeros(128, np.float32)
            for gi in range(min(gpt, n_g - t * gpt)):
                col[gi * o:(gi + 1) * o] = b[t * gpt + gi, :]
            bias_cols.append(col)
        return dict(nblk=nblk, idx=idx_arr, segs=segs, mms=mms,
                    ntile=ntile, gpt=gpt, o=o, bt0=bt0)

    for idx, W, b in ((idx1, W1, b1), (idx2, W2, b2), (idx3, W3, b3)):
        layers.append(pack_layer(np.asarray(idx), np.asarray(W),
                                 np.asarray(b), G, F, O))
    lo = pack_layer(np.asarray(idxo), np.asarray(Wo), np.asarray(bo),
                    GO_G, GO_F, GO_O)
    # merge x+h1 segments of the out layer (both ready after h1)
    wh = np.concatenate(wcols, axis=1)
    bias = np.stack(bias_cols, axis=1)  # [128, ncols]
    idx_all = np.concatenate([layers[0]["idx"], layers[1]["idx"],
                              layers[2]["idx"], lo["idx"]])
    return dict(layers=layers, lo=lo, wh=wh.astype(ml_dtypes.bfloat16),
                bias=bias.astype(np.float32),
                idx_wrapped=_wrap(idx_all), idx_total=idx_all.shape[0])


def _wrap(idx_list):
    n = idx_list.shape[0]
    w = idx_list.reshape(n // 16, 16).T
    return np.tile(w, (8, 1)).astype(np.int16)


def _apply_tile_patch():
    from concourse import tile as _tile
    from concourse.vector_clock import ScopedClock, VectorClock

    def _patched(self, tick_clock, wait_clock):
        nc = self.nc
        vc = tick_clock.global_clock
        for proc in range(len(vc)):
            tick = vc[proc]
            if tick > 0:
                nop_inst = nc.sync.nop(hint="drain_split_wait", nofuse=True)
                single = VectorClock()
                single.require_at_least(proc, tick)
                wait_clock.add_sem_waits(nop_inst.ins, ScopedClock({None: single}))
        nc.sync.drain()
        nc.all_engine_barrier()
        assert self.sems is not None
        popped = nc._tile_sem_poison_stack.pop()
        assert popped is self._sem_poison
        nc.clear_and_free_semaphores(list(self.sems.allocated().values()))
        nc.all_engine_barrier()

    _tile.TileContext._drain_and_barrier = _patched


def _build_program(plan, with_loop, force_q0=False):
    from concourse import bacc
    import concourse.mybir as mybir
    import concourse.tile as tile
    from concourse.masks import make_identity

    _apply_tile_patch()
    f32, bf16, i16 = mybir.dt.float32, mybir.dt.bfloat16, mybir.dt.int16
    L = plan["layers"]
    lo = plan["lo"]
    wh_cols = plan["wh"].shape[1]
    bias_ncol = plan["bias"].shape[1]
    idx_cols = plan["idx_total"] // 16

    nc = bacc.Bacc(
        "TRN2", target_bir_lowering=False, debug=False, num_devices=N_CORES,
        enable_asserts=False, num_swdge_queues=4,
        dynamic_dma_scratch_size=32768,
    )
    xt_in = nc.dram_tensor("xt", [IN, BS], bf16, kind="ExternalInput")
    idx_in = nc.dram_tensor("idxw", [128, idx_cols], i16, kind="ExternalInput")
    wh_in = nc.dram_tensor("wh", [128, wh_cols], bf16, kind="ExternalInput")
    bias_in = nc.dram_tensor("biasp", [128, bias_ncol], f32, kind="ExternalInput")
    y_out = nc.dram_tensor("y", [BS, 64], f32, kind="ExternalOutput")
    accT = nc.dram_tensor("accT", [HROWS, BS], bf16)
    if with_loop:
        nit_in = nc.dram_tensor("niter", [1, 1], mybir.dt.int32,
                                kind="ExternalInput")

    # idx column offset of each layer
    idx_off = [0]
    for lay in L:
        idx_off.append(idx_off[-1] + lay["nblk"] * 8)
    # queue schedule: (layer, cls) -> list of queues to split across
    qmap = {
        (0, 0): [0, 1, 2, 3], (1, 0): [0, 1], (2, 0): [0],
        (1, 1): [2, 3],
        (2, 1): [1],
        (2, 2): [0, 1],
        (3, 0): [2], (3, 1): [2], (3, 2): [3], (3, 3): [0],
    }

    with tile.TileContext(nc) as tc:
        with (
            tc.tile_pool(name="const", bufs=1) as consts,
            tc.tile_pool(name="ga", bufs=1) as gpa,
            tc.tile_pool(name="gb", bufs=1) as gpb,
            tc.tile_pool(name="hst", bufs=9) as hpool,
            tc.tile_pool(name="ps", bufs=8, space="PSUM") as pspool,
            tc.tile_pool(name="fin", bufs=1) as fpool,
        ):
            idx_t = consts.tile([128, idx_cols], i16)
            nc.sync.dma_start(out=idx_t[:], in_=idx_in[:])
            wh_t = consts.tile([128, wh_cols], bf16)
            nc.sync.dma_start(out=wh_t[:], in_=wh_in[:])
            bias_t = consts.tile([128, bias_ncol], f32)
            nc.sync.dma_start(out=bias_t[:], in_=bias_in[:])
            ident = consts.tile([128, 128], f32)
            make_identity(nc, ident)

            writes = {}  # h-layer (1..3) -> list of dma insts

            def emit_gathers(li, lay, gtile, classes):
                for (c, b0, b1, nidx) in lay["segs"]:
                    if c not in classes:
                        continue
                    qs = qmap.get((li, c), [c % 4])
                    nq = len(qs)
                    # split blocks across queues
                    tb = b1 - b0
                    per = (tb + nq - 1) // nq
                    for j, q in enumerate(qs):
                        sb = b0 + j * per
                        eb = min(b0 + (j + 1) * per, b1)
                        if sb >= eb:
                            continue
                        cnt = (eb - sb) * 128
                        c0 = idx_off[li] + sb * 8
                        gi = nc.gpsimd.dma_gather(
                            out_ap=gtile[:, sb:eb, :],
                            in_ap=(xt_in[:] if c == 0 else accT[:]),
                            idxs_ap=idx_t[:, c0:c0 + cnt // 16],
                            num_idxs=cnt,
                            num_idxs_reg=cnt,
                            elem_size=BS,
                            elem_step=BS,
                            single_packet=False,
                            queue_num=0 if force_q0 else q,
                        )
                        if c > 0:
                            for w in writes.get(c, []):
                                tile.add_dep_helper(gi.ins, w.ins, sync=True)

            def emit_compute(li, lay, gtile, hidden):
                """Returns list of h-write DMA insts (hidden) or None."""
                o = lay["o"]
                wl = []
                hstages = {}
                for ch in range(NCHUNK):
                    ps_tiles = []
                    for t in range(lay["ntile"]):
                        ps = pspool.tile([128, 512], f32, name="ps")
                        ps_tiles.append(ps)
                    prev_inst = {}
                    for mm in lay["mms"]:
                        mi = nc.tensor.matmul(
                            out=ps_tiles[mm.tile][mm.c0:mm.c1, :],
                            lhsT=wh_t[:, mm.w_off:mm.w_off + mm.w],
                            rhs=gtile[:, mm.blk, ch * 512:(ch + 1) * 512],
                            start=mm.start,
                            stop=mm.stop,
                            skip_group_check=True,
                            tile_position=(0, mm.c0),
                        )
                        if mm.tile in prev_inst:
                            tile.add_dep_helper(
                                mi.ins, prev_inst[mm.tile].ins, sync=False)
                        prev_inst[mm.tile] = mi
                    for t in range(lay["ntile"]):
                        m = min(lay["gpt"], (G if hidden else GO_G) - t * lay["gpt"]) * o
                        bcol = lay["bt0"] + t
                        if hidden:
                            if ch == 0:
                                hstages[t] = hpool.tile([128, BS], bf16,
                                                        name="hs")
                            hs = hstages[t]
                            nc.scalar.activation(
                                out=hs[0:m, ch * 512:(ch + 1) * 512],
                                in_=ps_tiles[t][0:m, :],
                                func=mybir.ActivationFunctionType.Tanh,
                                bias=bias_t[0:m, bcol:bcol + 1],
                                scale=1.0,
                            )
                            if ch == NCHUNK - 1:
                                r0 = li * G * O + t * 128
                                eng = nc.sync if t % 2 == 0 else nc.scalar
                                wr = eng.dma_start(
                                    out=accT[r0:r0 + m, :], in_=hs[0:m, :])
                                wl.append(wr)
                        else:
                            nc.vector.tensor_scalar_add(
                                out=yT[0:m, ch * 512:(ch + 1) * 512],
                                in0=ps_tiles[t][0:m, :],
                                scalar1=bias_t[0:m, bcol:bcol + 1],
                            )
                return wl

            def body(iv=None):
                writes.clear()
                gt = {}
                # t=0 gathers: L1 full (x-class) + L2 x-class
                gt[0] = gpa.tile([128, L[0]["nblk"], BS], bf16, name="gA")
                gt[1] = gpb.tile([128, L[1]["nblk"], BS], bf16, name="gB")
                emit_gathers(0, L[0], gt[0], {0})
                emit_gathers(1, L[1], gt[1], {0})

                global yT
                for li in range(3):
                    lay = L[li]
                    writes[li + 1] = emit_compute(li, lay, gt[li], hidden=True)
                    # prefetch notes: after emitting layer li's compute,
                    # emit gathers that depend on h_{li+1} or are prefetch
                    if li == 0:
                        # h1 ready classes: L2h1, L3x, L3h1
                        gt[2] = gpa.tile([128, L[2]["nblk"], BS], bf16,
                                         name="gA")
                        emit_gathers(1, L[1], gt[1], {1})
                        emit_gathers(2, L[2], gt[2], {0, 1})
                    elif li == 1:
                        gt[3] = gpb.tile([128, lo["nblk"], BS], bf16,
                                         name="gB")
                        emit_gathers(2, L[2], gt[2], {2})
                        emit_gathers(3, lo, gt[3], {0, 1, 2})
                    elif li == 2:
                        emit_gathers(3, lo, gt[3], {3})

                yT = fpool.tile([128, BS], f32, name="yT")
                emit_compute(3, lo, gt[3], hidden=False)
                ystage = fpool.tile([128, BS // 128, 64], f32, name="ystage")
                for c in range(BS // 128):
                    pst = pspool.tile([128, 512], f32, name="ps")
                    nc.tensor.transpose(
                        out=pst[0:128, 0:64],
                        in_=yT[0:64, c * 128:(c + 1) * 128],
                        identity=ident[0:64, 0:64],
                    )
                    nc.vector.tensor_copy(out=ystage[:, c, :],
                                          in_=pst[0:128, 0:64])
                nc.sync.dma_start(
                    out=y_out[:].rearrange("(c p) o -> p c o", p=128),
                    in_=ystage[:],
                )

            if with_loop:
                nit_t = consts.tile([1, 1], mybir.dt.int32)
                nc.sync.dma_start(out=nit_t[:], in_=nit_in[:])
                n = nc.values_load(nit_t[0:1, 0:1], min_val=0, max_val=2048,
                                   skip_runtime_bounds_check=True)
                with tc.For_i(0, n, 1):
                    body()
            else:
                body()

    nc.compile()
    return nc


class _Runner:
    """Persistent jitted SPMD executable (adapted from bass2jax)."""

    def __init__(self, nc):
        import jax
        import concourse.mybir as mybir
        from jax.sharding import Mesh, PartitionSpec
        from jax.experimental.shard_map import shard_map
        from concourse.bass2jax import (
            _bass_exec_p, partition_id_tensor, install_neuronx_cc_hook,
        )

        install_neuronx_cc_hook()
        self.jax = jax
        in_names, out_names, out_avals, zero_outs = [], [], [], []
        partition_name = (
            nc.partition_id_tensor.name if nc.partition_id_tensor else None
        )
        for alloc in nc.m.functions[0].allocations:
            if not isinstance(alloc, mybir.MemoryLocationSet):
                continue
            name = alloc.memorylocations[0].name
            if alloc.kind == "ExternalInput":
                if name != partition_name:
                    in_names.append(name)
            elif alloc.kind == "ExternalOutput":
                out_names.append(name)
                shape = tuple(alloc.tensor_shape)
                dtype = mybir.dt.np(alloc.dtype)
                out_avals.append(jax.core.ShapedArray(shape, dtype))
                zero_outs.append(np.zeros(shape, dtype))
        self.n_params = len(in_names)
        self.in_names = in_names[:]
        self.out_names = out_names
        self.out_avals = out_avals
        self.zero_outs = zero_outs
        all_in = in_names + out_names + ([partition_name] if partition_name else [])
        donate = tuple(range(self.n_params, self.n_params + len(out_names)))

        def _body(*args):
            operands = list(args)
            if partition_name is not None:
                operands.append(partition_id_tensor())
            return tuple(
                _bass_exec_p.bind(
                    *operands,
                    out_avals=tuple(out_avals),
                    in_names=tuple(all_in),
                    out_names=tuple(out_names),
                    lowering_input_output_aliases=(),
                    sim_require_finite=True,
                    sim_require_nnan=True,
                    nc=nc,
                )
            )

        devices = jax.devices()[:N_CORES]
        self.mesh = Mesh(np.asarray(devices), ("core",))
        self.sharded = jax.jit(
            shard_map(
                _body, mesh=self.mesh,
                in_specs=(PartitionSpec("core"),) * (self.n_params + len(out_names)),
                out_specs=(PartitionSpec("core"),) * len(out_names),
                check_rep=False,
            ),
            donate_argnums=donate,
            keep_unused=True,
        )

    def prep(self, in_maps, device_put=True):
        per_core = [[np.asarray(m[name]) for name in self.in_names] for m in in_maps]
        arrs = [
            np.concatenate([per_core[c][i] for c in range(N_CORES)], axis=0)
            for i in range(self.n_params)
        ]
        if device_put:
            from jax.sharding import NamedSharding, PartitionSpec

            sh = NamedSharding(self.mesh, PartitionSpec("core"))
            arrs = [self.jax.device_put(a, sh) for a in arrs]
            self.jax.block_until_ready(arrs)
        return arrs

    def run(self, concat_in):
        zeros = [
            np.zeros((N_CORES * z.shape[0], *z.shape[1:]), z.dtype)
            for z in self.zero_outs
        ]
        outs = self.sharded(*concat_in, *zeros)
        self.jax.block_until_ready(outs)
        return outs

    def split(self, out_arrs):
        return [
            {
                name: np.asarray(out_arrs[i]).reshape(
                    N_CORES, *self.out_avals[i].shape
                )[c]
                for i, name in enumerate(self.out_names)
            }
            for c in range(N_CORES)
        ]




def _get(plan_key, plan, with_loop):
    key = (plan_key, with_loop)
    if key not in _cache:
        nc = _build_program(plan, with_loop)
        _cache[key] = _Runner(nc)
    return _cache[key]


def _in_maps(plan, x, niter):
    bf = ml_dtypes.bfloat16
    x = np.asarray(x)
    maps = []
    for c in range(N_CORES):
        xs = x[c * BS:(c + 1) * BS, :]
        m = {
            "xt": np.ascontiguousarray(xs.T).astype(bf),
            "idxw": plan["idx_wrapped"],
            "wh": plan["wh"],
            "biasp": plan["bias"],
        }
        if niter is not None:
            m["niter"] = np.array([[niter]], np.int32)
        maps.append(m)
    return maps


def kernel(**inputs):
    niter = inputs.pop("_niter", None)
    x = inputs.pop("x")
    plan = build_plan(**{k: inputs[k] for k in (
        "idx1", "idx2", "idx3", "idxo", "W1", "W2", "W3", "Wo",
        "b1", "b2", "b3", "bo")})
    r = _get("p0", plan, niter is not None)
    ci = r.prep(_in_maps(plan, x, niter), device_put=False)
    outs = r.split(r.run(ci))
    return np.concatenate(
        [outs[c]["y"] for c in range(N_CORES)], axis=0).astype(np.float32)


def bench(inputs, k_hi=129, rounds=8, per=4):
    """On-device time per kernel-body iteration, measured as the median over
    interleaved A/B rounds of (wall(k_hi) - wall(1)) / (k_hi - 1)."""
    import time

    inputs = dict(inputs)
    x = inputs.pop("x")
    plan = build_plan(**{k: inputs[k] for k in (
        "idx1", "idx2", "idx3", "idxo", "W1", "W2", "W3", "Wo",
        "b1", "b2", "b3", "bo")})
    r = _get("p0", plan, True)
    ci1 = r.prep(_in_maps(plan, x, 1), device_put=True)
    cih = r.prep(_in_maps(plan, x, k_hi), device_put=True)
    outs = r.split(r.run(ci1))
    y1 = np.concatenate(
        [outs[c]["y"] for c in range(N_CORES)], axis=0).astype(np.float32)
    outs = r.split(r.run(cih))
    yh = np.concatenate(
        [outs[c]["y"] for c in range(N_CORES)], axis=0).astype(np.float32)
    diffs = []
    for _ in range(rounds):
        t1s, ths = [], []
        for _ in range(per):
            t0 = time.perf_counter(); r.run(ci1)
            t1s.append(time.perf_counter() - t0)
            t0 = time.perf_counter(); r.run(cih)
            ths.append(time.perf_counter() - t0)
        diffs.append((min(ths) - min(t1s)) / (k_hi - 1))
    diffs.sort()
    return diffs[len(diffs) // 2], y1, yh



# revision 2
# speedup vs baseline: 1.5428x; 1.5428x over previous
"""Trainium2 Bass kernel for nn_DirectEncodingModel (gnn_message_passing), v3.

Design vs baseline:
  - Dead-output pruning (backward fixed point): only h rows actually
    referenced downstream are computed/activated/stored. Used rows are
    packed into psum tiles of <=128 (tile boundaries on group boundaries),
    shrinking ACT + h-write + psum pressure (h1~827, h2~490, h3~183 of
    1024 each for random idx).
  - h stored fp8e4 in DRAM: h-class gathers and h-writes at half the
    bytes. x stays bf16 (fp8 x fails accuracy). Matmul runs mixed
    lhsT=bf16 stationary x rhs=fp8 moving (allowed; only fp32 must match).
  - Full-width stationaries: every matmul covers its psum tile's whole
    used width [0:u_t) -> exactly one matmul per (block, tile) with no
    buddy decomposition; PE moving passes drop ~2x.
  - X blocks: group-sorted slots packed 128/block (pad idx 0, zero
    weights). H blocks: tile-aligned so each is a single matmul.
  - S=2 batch halves pipelined: layer-boundary DRAM round trips of one
    half overlap compute of the other.
"""

import numpy as np
import ml_dtypes

B = 16384
IN = 512
G, F, O = 64, 24, 16
GO_G, GO_F, GO_O = 4, 48, 16
N_CORES = 8
BS = B // N_CORES          # 2048 per core
S = 2                      # batch halves per core
HB = BS // S               # 1024
NCH = HB // 512            # moving chunks per half
H_FP8 = True               # h storage dtype flag
MAX_GATHER_BLOCKS = 4      # blocks per dma_gather instruction

_cache = {}


class MM:
    __slots__ = ("kind", "blk", "tile", "w_off", "w", "start", "stop")

    def __init__(self, kind, blk, tile, w_off, w):
        self.kind, self.blk, self.tile = kind, blk, tile
        self.w_off, self.w = w_off, w
        self.start = False
        self.stop = False


def build_plan(idx1, idx2, idx3, idxo, W1, W2, W3, Wo, b1, b2, b3, bo):
    idxs = [np.asarray(a) for a in (idx1, idx2, idx3, idxo)]
    Ws = [np.asarray(a) for a in (W1, W2, W3, Wo)]
    bs = [np.asarray(a) for a in (b1, b2, b3, bo)]
    n_groups = [G, G, G, GO_G]
    fans = [F, F, F, GO_F]
    outs = [O, O, O, GO_O]

    # ---- backward pass: active groups + used h rows ----
    # used[l][r] for hidden layer l in 0..2 (h_{l+1} rows r in 0..1023)
    used = [np.zeros(G * O, bool) for _ in range(3)]
    active = [None, None, None, np.ones(GO_G, bool)]
    for li in (3, 2, 1):
        for g in range(n_groups[li]):
            if not active[li][g]:
                continue
            for v in idxs[li][g]:
                v = int(v)
                if v >= IN:
                    l = (v - IN) // (G * O)
                    used[l][(v - IN) % (G * O)] = True
        if li >= 1:
            lsrc = li - 1  # hidden layer index producing h_{li}
            act = np.zeros(G, bool)
            u = used[lsrc].reshape(G, O)
            act[u.any(axis=1)] = True
            active[lsrc] = act

    # ---- packed storage: tiles per layer (hidden 0..2 and out layer 3) ----
    # tiles[li] = list of dict(groups=[g..], cols={(g,o)->col}, u=width)
    layer_tiles = []
    pos_map = [dict() for _ in range(3)]  # (l, r) -> packed accT row
    acc_rows = 0
    layer_base = []
    for li in range(4):
        tiles = []
        cur = None
        ng = n_groups[li]
        for g in range(ng):
            if not active[li][g]:
                continue
            if li < 3:
                uo = [o for o in range(outs[li]) if used[li][g * O + o]]
            else:
                uo = list(range(GO_O))
            if not uo:
                continue
            if cur is None or cur["u"] + len(uo) > 128:
                cur = dict(groups=[], cols={}, u=0)
                tiles.append(cur)
            for o in uo:
                cur["cols"][(g, o)] = cur["u"]
                cur["u"] += 1
            cur["groups"].append(g)
        layer_tiles.append(tiles)
        layer_base.append(acc_rows)
        if li < 3:
            for t_i, t in enumerate(tiles):
                for (g, o), c in t["cols"].items():
                    pos_map[li][g * O + o] = acc_rows + c
                acc_rows += t["u"]
    acc_rows_total = acc_rows

    # tile index of each group per layer
    tile_of_group = []
    for li in range(4):
        tg = {}
        for t_i, t in enumerate(layer_tiles[li]):
            for g in t["groups"]:
                tg[g] = t_i
        tile_of_group.append(tg)

    # ---- per layer: X blocks, H blocks, gather lists, mms, bias ----
    wcols = []      # stationary column blocks [128, u]
    w_off_cum = [0]
    bias_cols = []  # [128] per (layer, tile)
    layers = []
    xg_lists = []   # per layer: X gather idx list (xt rows)
    hg_lists = []   # per layer: H gather idx list (accT rows)

    def woff():
        return w_off_cum[-1]

    for li in range(4):
        ng, fan, o = n_groups[li], fans[li], outs[li]
        tiles = layer_tiles[li]
        idx = idxs[li]
        W = Ws[li]
        # slot lists
        xslots, hslots = [], []
        for g in range(ng):
            if g not in tile_of_group[li]:
                continue
            for f in range(fan):
                v = int(idx[g, f])
                if v < IN:
                    xslots.append((g, f, v))
                else:
                    l = (v - IN) // (G * O)
                    r = (v - IN) % (G * O)
                    hslots.append((g, f, pos_map[l][r]))
        # X blocks: group-major, 128 per block, pad idx 0
        nblk_x = (len(xslots) + 127) // 128 if xslots else 0
        xg = np.zeros(nblk_x * 128, np.int64)
        for i, s in enumerate(xslots):
            xg[i] = s[2]
        # H blocks: tile-aligned
        h_by_tile = [[] for _ in tiles]
        for s in hslots:
            h_by_tile[tile_of_group[li][s[0]]].append(s)
        hblocks = []  # (tile, [slots])
        for t_i, sl in enumerate(h_by_tile):
            for b0 in range(0, len(sl), 128):
                hblocks.append((t_i, sl[b0:b0 + 128]))
        nblk_h = len(hblocks)
        hg = np.zeros(nblk_h * 128, np.int64)
        for b_i, (t_i, sl) in enumerate(hblocks):
            for j, s in enumerate(sl):
                hg[b_i * 128 + j] = s[2]
        # mms
        mms = []
        for b_i in range(nblk_x):
            bslots = xslots[b_i * 128:(b_i + 1) * 128]
            btiles = sorted({tile_of_group[li][s[0]] for s in bslots})
            for t_i in btiles:
                t = tiles[t_i]
                stat = np.zeros((128, t["u"]), np.float32)
                for p, (g, f, v) in enumerate(bslots):
                    if tile_of_group[li][g] != t_i:
                        continue
                    for (gg, oo), c in t["cols"].items():
                        if gg == g:
                            stat[p, c] = W[g, f, oo]
                wcols.append(stat)
                w_off_cum.append(woff() + t["u"])
                mms.append(MM("x", b_i, t_i, w_off_cum[-2], t["u"]))
        for b_i, (t_i, sl) in enumerate(hblocks):
            t = tiles[t_i]
            stat = np.zeros((128, t["u"]), np.float32)
            for p, (g, f, v) in enumerate(sl):
                for (gg, oo), c in t["cols"].items():
                    if gg == g:
                        stat[p, c] = W[g, f, oo]
            wcols.append(stat)
            w_off_cum.append(woff() + t["u"])
            mms.append(MM("h", b_i, t_i, w_off_cum[-2], t["u"]))
        # start/stop flags per tile
        first_seen, last_seen = {}, {}
        for i, m in enumerate(mms):
            if m.tile not in first_seen:
                first_seen[m.tile] = i
            last_seen[m.tile] = i
        for i in first_seen.values():
            mms[i].start = True
        for i in last_seen.values():
            mms[i].stop = True
        # bias
        bt0 = len(bias_cols)
        for t in tiles:
            col = np.zeros(128, np.float32)
            for (g, oo), c in t["cols"].items():
                col[c] = bs[li][g, oo]
            bias_cols.append(col)
        layers.append(dict(
            nblk_x=nblk_x, nblk_h=nblk_h, mms=mms, tiles=tiles, bt0=bt0,
        ))
        xg_lists.append(xg)
        hg_lists.append(hg)

    # ---- pack gather idx tensor ----
    # order: all X lists (per layer), then all H lists
    idx_all = []
    idx_off = {}
    cur = 0
    for li in range(4):
        idx_off[("x", li)] = cur
        idx_all.append(xg_lists[li])
        cur += len(xg_lists[li])
    for li in range(4):
        idx_off[("h", li)] = cur
        idx_all.append(hg_lists[li])
        cur += len(hg_lists[li])
    idx_cat = np.concatenate([a for a in idx_all if len(a)]) if cur else np.zeros(0)
    assert idx_cat.max(initial=0) < max(acc_rows_total, 1) + IN
    wh = np.concatenate(wcols, axis=1) if wcols else np.zeros((128, 0))
    bias = (np.stack(bias_cols, axis=1) if bias_cols
            else np.zeros((128, 0), np.float32))
    return dict(
        layers=layers, layer_tiles=layer_tiles, layer_base=layer_base,
        acc_rows=acc_rows_total,
        wh=wh.astype(ml_dtypes.bfloat16),
        bias=bias.astype(np.float32),
        idx_wrapped=_wrap(idx_cat.astype(np.int16)),
        idx_off=idx_off, idx_total=len(idx_cat),
    )


def _wrap(idx_list):
    n = idx_list.shape[0]
    if n == 0:
        return np.zeros((128, 1), np.int16)
    w = idx_list.reshape(n // 16, 16).T
    return np.tile(w, (8, 1)).astype(np.int16)


def _apply_tile_patch():
    from concourse import tile as _tile
    from concourse.vector_clock import ScopedClock, VectorClock

    def _patched(self, tick_clock, wait_clock):
        nc = self.nc
        vc = tick_clock.global_clock
        for proc in range(len(vc)):
            tick = vc[proc]
            if tick > 0:
                nop_inst = nc.sync.nop(hint="drain_split_wait", nofuse=True)
                single = VectorClock()
                single.require_at_least(proc, tick)
                wait_clock.add_sem_waits(nop_inst.ins, ScopedClock({None: single}))
        nc.sync.drain()
        nc.all_engine_barrier()
        assert self.sems is not None
        popped = nc._tile_sem_poison_stack.pop()
        assert popped is self._sem_poison
        nc.clear_and_free_semaphores(list(self.sems.allocated().values()))
        nc.all_engine_barrier()

    _tile.TileContext._drain_and_barrier = _patched


def _build_program(plan, with_loop):
    from concourse import bacc
    import concourse.mybir as mybir
    import concourse.tile as tile
    from concourse.masks import make_identity

    _apply_tile_patch()
    f32, bf16, i16 = mybir.dt.float32, mybir.dt.bfloat16, mybir.dt.int16
    f8 = mybir.dt.float8e4 if H_FP8 else bf16
    L = plan["layers"]
    wh_cols = max(plan["wh"].shape[1], 1)
    bias_ncol = max(plan["bias"].shape[1], 1)
    idx_cols = plan["idx_wrapped"].shape[1]
    acc_rows = plan["acc_rows"]
    layer_base = plan["layer_base"]

    nc = bacc.Bacc(
        "TRN2", target_bir_lowering=False, debug=False, num_devices=N_CORES,
        enable_asserts=False, num_swdge_queues=4,
        dynamic_dma_scratch_size=32768,
    )
    xt_in = nc.dram_tensor("xt", [IN, BS], bf16, kind="ExternalInput")
    idx_in = nc.dram_tensor("idxw", [128, idx_cols], i16, kind="ExternalInput")
    wh_in = nc.dram_tensor("wh", [128, wh_cols], bf16, kind="ExternalInput")
    bias_in = nc.dram_tensor("biasp", [128, bias_ncol], f32, kind="ExternalInput")
    y_out = nc.dram_tensor("y", [BS, 64], f32, kind="ExternalOutput")
    accT = [nc.dram_tensor(f"accT{hf}", [max(acc_rows, 1), HB], f8)
            for hf in range(S)]
    if with_loop:
        nit_in = nc.dram_tensor("niter", [1, 1], mybir.dt.int32,
                                kind="ExternalInput")

    with tile.TileContext(nc) as tc:
        with (
            tc.tile_pool(name="const", bufs=1) as consts,
            tc.tile_pool(name="gx", bufs=1) as gxpool,
            tc.tile_pool(name="gh", bufs=1) as ghpool,
            tc.tile_pool(name="hst", bufs=1) as hpool,
            tc.tile_pool(name="ps", bufs=8, space="PSUM") as pspool,
            tc.tile_pool(name="fin", bufs=1) as fpool,
        ):
            idx_t = consts.tile([128, idx_cols], i16)
            nc.sync.dma_start(out=idx_t[:], in_=idx_in[:])
            wh_t = consts.tile([128, wh_cols], bf16)
            nc.sync.dma_start(out=wh_t[:], in_=wh_in[:])
            bias_t = consts.tile([128, bias_ncol], f32)
            nc.sync.dma_start(out=bias_t[:], in_=bias_in[:])
            ident = consts.tile([128, 128], f32)
            make_identity(nc, ident)

            qrr = [0]

            def next_q():
                q = qrr[0] % 4
                qrr[0] += 1
                return q

            def emit_gather(kind, li, hf, gtile, writes=None):
                """Gather all blocks of (kind, li) for half hf into gtile.

                Split into <=MAX_GATHER_BLOCKS-block pieces across queues.
                """
                nblk = L[li]["nblk_x"] if kind == "x" else L[li]["nblk_h"]
                if nblk == 0:
                    return
                off0 = plan["idx_off"][(kind, li)]
                for b0 in range(0, nblk, MAX_GATHER_BLOCKS):
                    b1 = min(b0 + MAX_GATHER_BLOCKS, nblk)
                    cnt = (b1 - b0) * 128
                    c0 = (off0 + b0 * 128) // 16
                    if kind == "x":
                        in_ap = xt_in[:, hf * HB:(hf + 1) * HB]
                        es = 2048
                    else:
                        in_ap = accT[hf][:]
                        es = None
                    gi = nc.gpsimd.dma_gather(
                        out_ap=gtile[:, b0:b1, :],
                        in_ap=in_ap,
                        idxs_ap=idx_t[:, c0:c0 + cnt // 16],
                        num_idxs=cnt,
                        num_idxs_reg=cnt,
                        elem_size=HB,
                        elem_step=es,
                        single_packet=False,
                        queue_num=next_q(),
                    )
                    if writes:
                        for w in writes:
                            tile.add_dep_helper(gi.ins, w.ins, sync=True)

            def emit_layer(li, hf, gx, gh, hw_writes):
                """mms + ACT + h-writes for (layer li, half hf).

                Returns list of h-write DMA insts (hidden layers)."""
                lay = L[li]
                tiles = lay["tiles"]
                ntile = len(tiles)
                hidden = li < 3
                wl = []
                hstages = {}
                for ch in range(NCH):
                    ps_tiles = [pspool.tile([128, 512], f32, name="ps")
                                for _ in range(ntile)]
                    prev_inst = {}
                    for mm in lay["mms"]:
                        gt = gx if mm.kind == "x" else gh
                        mi = nc.tensor.matmul(
                            out=ps_tiles[mm.tile][0:mm.w, :],
                            lhsT=wh_t[:, mm.w_off:mm.w_off + mm.w],
                            rhs=gt[:, mm.blk, ch * 512:(ch + 1) * 512],
                            start=mm.start,
                            stop=mm.stop,
                            skip_group_check=True,
                        )
                        if mm.tile in prev_inst:
                            tile.add_dep_helper(
                                mi.ins, prev_inst[mm.tile].ins, sync=False)
                        prev_inst[mm.tile] = mi
                    for t_i in range(ntile):
                        u = tiles[t_i]["u"]
                        bcol = lay["bt0"] + t_i
                        if hidden:
                            if ch == 0:
                                hstages[t_i] = hpool.tile(
                                    [128, HB], f8, name="hs",
                                    tag=f"hs{li}_{t_i}_{hf}")
                            hs = hstages[t_i]
                            nc.scalar.activation(
                                out=hs[0:u, ch * 512:(ch + 1) * 512],
                                in_=ps_tiles[t_i][0:u, :],
                                func=mybir.ActivationFunctionType.Tanh,
                                bias=bias_t[0:u, bcol:bcol + 1],
                                scale=1.0,
                            )
                            if ch == NCH - 1:
                                r0 = layer_base[li] + sum(
                                    tt["u"] for tt in tiles[:t_i])
                                eng = nc.sync if t_i % 2 == 0 else nc.scalar
                                wr = eng.dma_start(
                                    out=accT[hf][r0:r0 + u, :],
                                    in_=hs[0:u, :])
                                wl.append(wr)
                        else:
                            nc.vector.tensor_scalar_add(
                                out=yT[0:u, ch * 512:(ch + 1) * 512],
                                in0=ps_tiles[t_i][0:u, :],
                                scalar1=bias_t[0:u, bcol:bcol + 1],
                            )
                return wl

            def body(iv=None):
                global yT
                gx = {}
                gh = {}
                for hf in range(S):
                    for li in range(4):
                        if L[li]["nblk_x"]:
                            gx[(li, hf)] = gxpool.tile(
                                [128, L[li]["nblk_x"], HB], bf16, name="gx",
                                tag=f"gx{li}_{hf}")
                        if L[li]["nblk_h"]:
                            gh[(li, hf)] = ghpool.tile(
                                [128, L[li]["nblk_h"], HB], f8, name="gh",
                                tag=f"gh{li}_{hf}")
                # X gathers upfront, L1 first, halves interleaved
                for li in range(4):
                    for hf in range(S):
                        if (li, hf) in gx:
                            emit_gather("x", li, hf, gx[(li, hf)])
                writes = {0: [], 1: []}
                ys = []
                for li in range(4):
                    for hf in range(S):
                        if li == 3:
                            yT = fpool.tile([128, HB], f32, name="yT",
                                            tag=f"yT{hf}")
                        wl = emit_layer(li, hf, gx.get((li, hf)),
                                        gh.get((li, hf)), writes[hf])
                        writes[hf].extend(wl)
                        # H gather for next layer of this half
                        nli = li + 1
                        if nli <= 3 and (nli, hf) in gh:
                            emit_gather("h", nli, hf, gh[(nli, hf)],
                                        writes=writes[hf])
                        if li == 3:
                            # y epilogue for this half
                            ystage = fpool.tile([128, HB // 128, 64], f32,
                                                name="ystage",
                                                tag=f"ys{hf}")
                            for c in range(HB // 128):
                                pst = pspool.tile([128, 512], f32, name="ps")
                                nc.tensor.transpose(
                                    out=pst[0:128, 0:64],
                                    in_=yT[0:64, c * 128:(c + 1) * 128],
                                    identity=ident[0:64, 0:64],
                                )
                                nc.vector.tensor_copy(out=ystage[:, c, :],
                                                      in_=pst[0:128, 0:64])
                            nc.sync.dma_start(
                                out=y_out[hf * HB:(hf + 1) * HB, :].rearrange(
                                    "(c p) o -> p c o", p=128),
                                in_=ystage[:],
                            )

            if with_loop:
                nit_t = consts.tile([1, 1], mybir.dt.int32)
                nc.sync.dma_start(out=nit_t[:], in_=nit_in[:])
                n = nc.values_load(nit_t[0:1, 0:1], min_val=0, max_val=2048,
                                   skip_runtime_bounds_check=True)
                with tc.For_i(0, n, 1):
                    body()
            else:
                body()

    # Align each gather's SWDGE queue with its Tile-assigned DMASW sem lane
    # (sem->queue is locked 1:1 by the runtime; the scheduler reorders
    # instructions, so emission-order round-robin desyncs).
    from concourse.tile_scheduler import PROC_NAME_TO_IDX

    sw_procs = {PROC_NAME_TO_IDX[f"DMASW{i}"]: i for i in range(8)}

    def _fix_queues(blocks):
        for blk in blocks:
            for inst in blk.instructions:
                if isinstance(inst, mybir.InstDMAGatherAnt):
                    proc = getattr(inst, "bass_scheduled_proc", None)
                    if proc in sw_procs:
                        inst.queue_num = sw_procs[proc] % 4

    _fix_queues(nc.m.functions[0].blocks)
    nc.compile()
    return nc


# ---- runner (same as baseline) ----
class _Runner:
    def __init__(self, nc):
        import jax
        import concourse.mybir as mybir
        from jax.sharding import Mesh, PartitionSpec
        from jax.experimental.shard_map import shard_map
        from concourse.bass2jax import (
            _bass_exec_p, partition_id_tensor, install_neuronx_cc_hook,
        )

        install_neuronx_cc_hook()
        self.jax = jax
        in_names, out_names, out_avals, zero_outs = [], [], [], []
        partition_name = (
            nc.partition_id_tensor.name if nc.partition_id_tensor else None
        )
        for alloc in nc.m.functions[0].allocations:
            if not isinstance(alloc, mybir.MemoryLocationSet):
                continue
            name = alloc.memorylocations[0].name
            if alloc.kind == "ExternalInput":
                if name != partition_name:
                    in_names.append(name)
            elif alloc.kind == "ExternalOutput":
                out_names.append(name)
                shape = tuple(alloc.tensor_shape)
                dtype = mybir.dt.np(alloc.dtype)
                out_avals.append(jax.core.ShapedArray(shape, dtype))
                zero_outs.append(np.zeros(shape, dtype))
        self.n_params = len(in_names)
        self.in_names = in_names[:]
        self.out_names = out_names
        self.out_avals = out_avals
        self.zero_outs = zero_outs
        all_in = in_names + out_names + ([partition_name] if partition_name else [])
        donate = tuple(range(self.n_params, self.n_params + len(out_names)))

        def _body(*args):
            operands = list(args)
            if partition_name is not None:
                operands.append(partition_id_tensor())
            return tuple(
                _bass_exec_p.bind(
                    *operands,
                    out_avals=tuple(out_avals),
                    in_names=tuple(all_in),
                    out_names=tuple(out_names),
                    lowering_input_output_aliases=(),
                    sim_require_finite=True,
                    sim_require_nnan=True,
                    nc=nc,
                )
            )

        devices = jax.devices()[:N_CORES]
        self.mesh = Mesh(np.asarray(devices), ("core",))
        self.sharded = jax.jit(
            shard_map(
                _body, mesh=self.mesh,
                in_specs=(PartitionSpec("core"),) * (self.n_params + len(out_names)),
                out_specs=(PartitionSpec("core"),) * len(out_names),
                check_rep=False,
            ),
            donate_argnums=donate,
            keep_unused=True,
        )

    def prep(self, in_maps, device_put=True):
        per_core = [[np.asarray(m[name]) for name in self.in_names] for m in in_maps]
        arrs = [
            np.concatenate([per_core[c][i] for c in range(N_CORES)], axis=0)
            for i in range(self.n_params)
        ]
        if device_put:
            from jax.sharding import NamedSharding, PartitionSpec

            sh = NamedSharding(self.mesh, PartitionSpec("core"))
            arrs = [self.jax.device_put(a, sh) for a in arrs]
            self.jax.block_until_ready(arrs)
        return arrs

    def run(self, concat_in):
        zeros = [
            np.zeros((N_CORES * z.shape[0], *z.shape[1:]), z.dtype)
            for z in self.zero_outs
        ]
        outs = self.sharded(*concat_in, *zeros)
        self.jax.block_until_ready(outs)
        return outs

    def split(self, out_arrs):
        return [
            {
                name: np.asarray(out_arrs[i]).reshape(
                    N_CORES, *self.out_avals[i].shape
                )[c]
                for i, name in enumerate(self.out_names)
            }
            for c in range(N_CORES)
        ]


def _get(plan_key, plan, with_loop):
    key = (plan_key, with_loop)
    if key not in _cache:
        nc = _build_program(plan, with_loop)
        _cache[key] = _Runner(nc)
    return _cache[key]


def _in_maps(plan, x, niter):
    bf = ml_dtypes.bfloat16
    x = np.asarray(x)
    maps = []
    for c in range(N_CORES):
        xs = x[c * BS:(c + 1) * BS, :]
        m = {
            "xt": np.ascontiguousarray(xs.T).astype(bf),
            "idxw": plan["idx_wrapped"],
            "wh": plan["wh"],
            "biasp": plan["bias"],
        }
        if niter is not None:
            m["niter"] = np.array([[niter]], np.int32)
        maps.append(m)
    return maps


def kernel(**inputs):
    niter = inputs.pop("_niter", None)
    x = inputs.pop("x")
    plan = build_plan(**{k: inputs[k] for k in (
        "idx1", "idx2", "idx3", "idxo", "W1", "W2", "W3", "Wo",
        "b1", "b2", "b3", "bo")})
    r = _get("p0", plan, niter is not None)
    ci = r.prep(_in_maps(plan, x, niter), device_put=False)
    outs = r.split(r.run(ci))
    return np.concatenate(
        [outs[c]["y"] for c in range(N_CORES)], axis=0).astype(np.float32)


def bench(inputs, k_hi=129, rounds=8, per=4):
    import time

    inputs = dict(inputs)
    x = inputs.pop("x")
    plan = build_plan(**{k: inputs[k] for k in (
        "idx1", "idx2", "idx3", "idxo", "W1", "W2", "W3", "Wo",
        "b1", "b2", "b3", "bo")})
    r = _get("p0", plan, True)
    ci1 = r.prep(_in_maps(plan, x, 1), device_put=True)
    cih = r.prep(_in_maps(plan, x, k_hi), device_put=True)
    outs = r.split(r.run(ci1))
    y1 = np.concatenate(
        [outs[c]["y"] for c in range(N_CORES)], axis=0).astype(np.float32)
    outs = r.split(r.run(cih))
    yh = np.concatenate(
        [outs[c]["y"] for c in range(N_CORES)], axis=0).astype(np.float32)
    diffs = []
    for _ in range(rounds):
        t1s, ths = [], []
        for _ in range(per):
            t0 = time.perf_counter(); r.run(ci1)
            t1s.append(time.perf_counter() - t0)
            t0 = time.perf_counter(); r.run(cih)
            ths.append(time.perf_counter() - t0)
        diffs.append((min(ths) - min(t1s)) / (k_hi - 1))
    diffs.sort()
    return diffs[len(diffs) // 2], y1, yh


# revision 3
# speedup vs baseline: 2.0956x; 1.3583x over previous
"""Trainium2 Bass kernel for nn_DirectEncodingModel (gnn_message_passing), v3.

Design vs baseline:
  - Dead-output pruning (backward fixed point): only h rows actually
    referenced downstream are computed/activated/stored. Used rows are
    packed into psum tiles of <=128 (tile boundaries on group boundaries),
    shrinking ACT + h-write + psum pressure (h1~827, h2~490, h3~183 of
    1024 each for random idx).
  - h stored fp8e4 in DRAM: h-class gathers and h-writes at half the
    bytes. x stays bf16 (fp8 x fails accuracy). Matmul runs mixed
    lhsT=bf16 stationary x rhs=fp8 moving (allowed; only fp32 must match).
  - Full-width stationaries: every matmul covers its psum tile's whole
    used width [0:u_t) -> exactly one matmul per (block, tile) with no
    buddy decomposition; PE moving passes drop ~2x.
  - X blocks: group-sorted slots packed 128/block (pad idx 0, zero
    weights). H blocks: tile-aligned so each is a single matmul.
  - S=2 batch halves pipelined: layer-boundary DRAM round trips of one
    half overlap compute of the other.
"""

import numpy as np
import ml_dtypes

B = 16384
IN = 512
G, F, O = 64, 24, 16
GO_G, GO_F, GO_O = 4, 48, 16
N_CORES = 8
BS = B // N_CORES          # 2048 per core
S = 2                      # batch halves per core
HB = BS // S               # 1024
NCH = HB // 512            # moving chunks per half
H_FP8 = True               # h storage dtype flag
MAX_GATHER_BLOCKS = 6      # blocks per dma_gather instruction
import os
PS_BUFS = tuple(int(v) for v in os.environ.get(
    "PS_BUFS", "3,3,1,1").split(","))  # PSUM banks per layer (sum <= 8)

_cache = {}


class MM:
    __slots__ = ("kind", "blk", "tile", "w_off", "w", "start", "stop")

    def __init__(self, kind, blk, tile, w_off, w):
        self.kind, self.blk, self.tile = kind, blk, tile
        self.w_off, self.w = w_off, w
        self.start = False
        self.stop = False


def build_plan(idx1, idx2, idx3, idxo, W1, W2, W3, Wo, b1, b2, b3, bo):
    idxs = [np.asarray(a) for a in (idx1, idx2, idx3, idxo)]
    Ws = [np.asarray(a) for a in (W1, W2, W3, Wo)]
    bs = [np.asarray(a) for a in (b1, b2, b3, bo)]
    n_groups = [G, G, G, GO_G]
    fans = [F, F, F, GO_F]
    outs = [O, O, O, GO_O]

    # ---- backward pass: active groups + used h rows ----
    # used[l][r] for hidden layer l in 0..2 (h_{l+1} rows r in 0..1023)
    used = [np.zeros(G * O, bool) for _ in range(3)]
    active = [None, None, None, np.ones(GO_G, bool)]
    for li in (3, 2, 1):
        for g in range(n_groups[li]):
            if not active[li][g]:
                continue
            for v in idxs[li][g]:
                v = int(v)
                if v >= IN:
                    l = (v - IN) // (G * O)
                    used[l][(v - IN) % (G * O)] = True
        if li >= 1:
            lsrc = li - 1  # hidden layer index producing h_{li}
            act = np.zeros(G, bool)
            u = used[lsrc].reshape(G, O)
            act[u.any(axis=1)] = True
            active[lsrc] = act

    # ---- packed storage: tiles per layer (hidden 0..2 and out layer 3) ----
    # tiles[li] = list of dict(groups=[g..], cols={(g,o)->col}, u=width)
    layer_tiles = []
    pos_map = [dict() for _ in range(3)]  # (l, r) -> packed accT row
    acc_rows = 0
    layer_base = []
    for li in range(4):
        tiles = []
        cur = None
        ng = n_groups[li]
        for g in range(ng):
            if not active[li][g]:
                continue
            if li < 3:
                uo = [o for o in range(outs[li]) if used[li][g * O + o]]
            else:
                uo = list(range(GO_O))
            if not uo:
                continue
            if cur is None or cur["u"] + len(uo) > 128:
                cur = dict(groups=[], cols={}, u=0)
                tiles.append(cur)
            for o in uo:
                cur["cols"][(g, o)] = cur["u"]
                cur["u"] += 1
            cur["groups"].append(g)
        layer_tiles.append(tiles)
        layer_base.append(acc_rows)
        if li < 3:
            for t_i, t in enumerate(tiles):
                for (g, o), c in t["cols"].items():
                    pos_map[li][g * O + o] = acc_rows + c
                acc_rows += t["u"]
    acc_rows_total = acc_rows

    # tile index of each group per layer
    tile_of_group = []
    for li in range(4):
        tg = {}
        for t_i, t in enumerate(layer_tiles[li]):
            for g in t["groups"]:
                tg[g] = t_i
        tile_of_group.append(tg)

    # ---- per layer: X blocks, H blocks, gather lists, mms, bias ----
    wcols = []      # stationary column blocks [128, u]
    w_off_cum = [0]
    bias_cols = []  # [128] per (layer, tile)
    layers = []
    xg_lists = []   # per layer: X gather idx list (xt rows)
    hg_lists = []   # per layer: H gather idx list (accT rows)

    def woff():
        return w_off_cum[-1]

    for li in range(4):
        ng, fan, o = n_groups[li], fans[li], outs[li]
        tiles = layer_tiles[li]
        idx = idxs[li]
        W = Ws[li]
        # slot lists
        xslots, hslots = [], []
        for g in range(ng):
            if g not in tile_of_group[li]:
                continue
            for f in range(fan):
                v = int(idx[g, f])
                if v < IN:
                    xslots.append((g, f, v))
                else:
                    l = (v - IN) // (G * O)
                    r = (v - IN) % (G * O)
                    hslots.append((g, f, pos_map[l][r]))
        # X blocks: group-major, 128 per block, pad idx 0
        nblk_x = (len(xslots) + 127) // 128 if xslots else 0
        xg = np.zeros(nblk_x * 128, np.int64)
        for i, s in enumerate(xslots):
            xg[i] = s[2]
        # H blocks: tile-aligned
        h_by_tile = [[] for _ in tiles]
        for s in hslots:
            h_by_tile[tile_of_group[li][s[0]]].append(s)
        hblocks = []  # (tile, [slots])
        for t_i, sl in enumerate(h_by_tile):
            for b0 in range(0, len(sl), 128):
                hblocks.append((t_i, sl[b0:b0 + 128]))
        nblk_h = len(hblocks)
        hg = np.zeros(nblk_h * 128, np.int64)
        for b_i, (t_i, sl) in enumerate(hblocks):
            for j, s in enumerate(sl):
                hg[b_i * 128 + j] = s[2]
        # mms
        mms = []
        for b_i in range(nblk_x):
            bslots = xslots[b_i * 128:(b_i + 1) * 128]
            btiles = sorted({tile_of_group[li][s[0]] for s in bslots})
            for t_i in btiles:
                t = tiles[t_i]
                stat = np.zeros((128, t["u"]), np.float32)
                for p, (g, f, v) in enumerate(bslots):
                    if tile_of_group[li][g] != t_i:
                        continue
                    for (gg, oo), c in t["cols"].items():
                        if gg == g:
                            stat[p, c] = W[g, f, oo]
                wcols.append(stat)
                w_off_cum.append(woff() + t["u"])
                mms.append(MM("x", b_i, t_i, w_off_cum[-2], t["u"]))
        for b_i, (t_i, sl) in enumerate(hblocks):
            t = tiles[t_i]
            stat = np.zeros((128, t["u"]), np.float32)
            for p, (g, f, v) in enumerate(sl):
                for (gg, oo), c in t["cols"].items():
                    if gg == g:
                        stat[p, c] = W[g, f, oo]
            wcols.append(stat)
            w_off_cum.append(woff() + t["u"])
            mms.append(MM("h", b_i, t_i, w_off_cum[-2], t["u"]))
        # start/stop flags per tile
        first_seen, last_seen = {}, {}
        for i, m in enumerate(mms):
            if m.tile not in first_seen:
                first_seen[m.tile] = i
            last_seen[m.tile] = i
        for i in first_seen.values():
            mms[i].start = True
        for i in last_seen.values():
            mms[i].stop = True
        # bias
        bt0 = len(bias_cols)
        for t in tiles:
            col = np.zeros(128, np.float32)
            for (g, oo), c in t["cols"].items():
                col[c] = bs[li][g, oo]
            bias_cols.append(col)
        layers.append(dict(
            nblk_x=nblk_x, nblk_h=nblk_h, mms=mms, tiles=tiles, bt0=bt0,
        ))
        xg_lists.append(xg)
        hg_lists.append(hg)

    # ---- pack gather idx tensor ----
    # order: all X lists (per layer), then all H lists
    idx_all = []
    idx_off = {}
    cur = 0
    for li in range(4):
        idx_off[("x", li)] = cur
        idx_all.append(xg_lists[li])
        cur += len(xg_lists[li])
    for li in range(4):
        idx_off[("h", li)] = cur
        idx_all.append(hg_lists[li])
        cur += len(hg_lists[li])
    idx_cat = np.concatenate([a for a in idx_all if len(a)]) if cur else np.zeros(0)
    assert idx_cat.max(initial=0) < max(acc_rows_total, 1) + IN
    wh = np.concatenate(wcols, axis=1) if wcols else np.zeros((128, 0))
    bias = (np.stack(bias_cols, axis=1) if bias_cols
            else np.zeros((128, 0), np.float32))
    return dict(
        layers=layers, layer_tiles=layer_tiles, layer_base=layer_base,
        acc_rows=acc_rows_total, bo_flat=np.asarray(bs[3]).reshape(64),
        wh=wh.astype(ml_dtypes.bfloat16),
        bias=bias.astype(np.float32),
        idx_wrapped=_wrap(idx_cat.astype(np.int16)),
        idx_off=idx_off, idx_total=len(idx_cat),
    )


def _wrap(idx_list):
    n = idx_list.shape[0]
    if n == 0:
        return np.zeros((128, 1), np.int16)
    w = idx_list.reshape(n // 16, 16).T
    return np.tile(w, (8, 1)).astype(np.int16)


def _apply_tile_patch():
    from concourse import tile as _tile
    from concourse.vector_clock import ScopedClock, VectorClock

    def _patched(self, tick_clock, wait_clock):
        nc = self.nc
        vc = tick_clock.global_clock
        for proc in range(len(vc)):
            tick = vc[proc]
            if tick > 0:
                nop_inst = nc.sync.nop(hint="drain_split_wait", nofuse=True)
                single = VectorClock()
                single.require_at_least(proc, tick)
                wait_clock.add_sem_waits(nop_inst.ins, ScopedClock({None: single}))
        nc.sync.drain()
        nc.all_engine_barrier()
        assert self.sems is not None
        popped = nc._tile_sem_poison_stack.pop()
        assert popped is self._sem_poison
        nc.clear_and_free_semaphores(list(self.sems.allocated().values()))
        nc.all_engine_barrier()

    _tile.TileContext._drain_and_barrier = _patched


def _build_program(plan, with_loop):
    from concourse import bacc
    import concourse.mybir as mybir
    import concourse.tile as tile
    from concourse.masks import make_identity

    _apply_tile_patch()
    f32, bf16, i16 = mybir.dt.float32, mybir.dt.bfloat16, mybir.dt.int16
    f8 = mybir.dt.float8e4 if H_FP8 else bf16
    L = plan["layers"]
    wh_cols = max(plan["wh"].shape[1], 1)
    bias_ncol = max(plan["bias"].shape[1], 1)
    idx_cols = plan["idx_wrapped"].shape[1]
    acc_rows = plan["acc_rows"]
    layer_base = plan["layer_base"]

    nc = bacc.Bacc(
        "TRN2", target_bir_lowering=False, debug=False, num_devices=N_CORES,
        enable_asserts=False, num_swdge_queues=4,
        dynamic_dma_scratch_size=32768,
    )
    xt_in = nc.dram_tensor("xt", [IN, BS], bf16, kind="ExternalInput")
    idx_in = nc.dram_tensor("idxw", [128, idx_cols], i16, kind="ExternalInput")
    wh_in = nc.dram_tensor("wh", [128, wh_cols], bf16, kind="ExternalInput")
    bias_in = nc.dram_tensor("biasp", [128, bias_ncol], f32, kind="ExternalInput")
    biaso_in = nc.dram_tensor("biaso", [128, 64], f32, kind="ExternalInput")
    y_out = nc.dram_tensor("y", [BS, 64], f32, kind="ExternalOutput")
    accT = [nc.dram_tensor(f"accT{hf}", [max(acc_rows, 1), HB], f8)
            for hf in range(S)]
    if with_loop:
        nit_in = nc.dram_tensor("niter", [1, 1], mybir.dt.int32,
                                kind="ExternalInput")

    with tile.TileContext(nc) as tc:
        with (
            tc.tile_pool(name="const", bufs=1) as consts,
            tc.tile_pool(name="gx", bufs=1) as gxpool,
            tc.tile_pool(name="gh", bufs=1) as ghpool,
            tc.tile_pool(name="hst", bufs=1) as hpool,
            tc.tile_pool(name="ps", bufs=8, space="PSUM") as pspool,
            tc.tile_pool(name="fin", bufs=1) as fpool,
        ):
            idx_t = consts.tile([128, idx_cols], i16)
            nc.sync.dma_start(out=idx_t[:], in_=idx_in[:])
            wh_t = consts.tile([128, wh_cols], bf16)
            nc.sync.dma_start(out=wh_t[:], in_=wh_in[:])
            bias_t = consts.tile([128, bias_ncol], f32)
            nc.sync.dma_start(out=bias_t[:], in_=bias_in[:])
            biaso_t = consts.tile([128, 64], f32)
            nc.sync.dma_start(out=biaso_t[:], in_=biaso_in[:])

            qrr = [0]

            def next_q():
                q = qrr[0] % 4
                qrr[0] += 1
                return q

            def emit_gather(kind, li, hf, gtile, writes=None):
                """Gather all blocks of (kind, li) for half hf into gtile.

                Split into <=MAX_GATHER_BLOCKS-block pieces across queues.
                """
                nblk = L[li]["nblk_x"] if kind == "x" else L[li]["nblk_h"]
                if nblk == 0:
                    return
                off0 = plan["idx_off"][(kind, li)]
                for b0 in range(0, nblk, MAX_GATHER_BLOCKS):
                    b1 = min(b0 + MAX_GATHER_BLOCKS, nblk)
                    cnt = (b1 - b0) * 128
                    c0 = (off0 + b0 * 128) // 16
                    if kind == "x":
                        in_ap = xt_in[:, hf * HB:(hf + 1) * HB]
                        es = 2048
                    else:
                        in_ap = accT[hf][:]
                        es = None
                    gi = nc.gpsimd.dma_gather(
                        out_ap=gtile[:, b0:b1, :],
                        in_ap=in_ap,
                        idxs_ap=idx_t[:, c0:c0 + cnt // 16],
                        num_idxs=cnt,
                        num_idxs_reg=cnt,
                        elem_size=HB,
                        elem_step=es,
                        single_packet=False,
                        queue_num=next_q(),
                    )
                    if writes:
                        for w in writes:
                            tile.add_dep_helper(gi.ins, w.ins, sync=True)

            def emit_layer(li, hf, gx, gh):
                """mms + ACT + h-writes for hidden layer (li, half hf).

                Returns list of h-write DMA insts."""
                lay = L[li]
                tiles = lay["tiles"]
                ntile = len(tiles)
                wl = []
                hstages = {}
                for ch in range(NCH):
                    ps_tiles = [pspool.tile([128, 512], f32, name="ps")
                                for _ in range(ntile)]
                    prev_inst = {}
                    for mm in lay["mms"]:
                        gt = gx if mm.kind == "x" else gh
                        mi = nc.tensor.matmul(
                            out=ps_tiles[mm.tile][0:mm.w, :],
                            lhsT=wh_t[:, mm.w_off:mm.w_off + mm.w],
                            rhs=gt[:, mm.blk, ch * 512:(ch + 1) * 512],
                            start=mm.start,
                            stop=mm.stop,
                            skip_group_check=True,
                        )
                        if mm.tile in prev_inst:
                            tile.add_dep_helper(
                                mi.ins, prev_inst[mm.tile].ins, sync=False)
                        prev_inst[mm.tile] = mi
                    for t_i in range(ntile):
                        u = tiles[t_i]["u"]
                        bcol = lay["bt0"] + t_i
                        if ch == 0:
                            hstages[t_i] = hpool.tile(
                                [128, HB], f8, name="hs",
                                tag=f"hs{li}_{t_i}_{hf}")
                        hs = hstages[t_i]
                        nc.scalar.activation(
                            out=hs[0:u, ch * 512:(ch + 1) * 512],
                            in_=ps_tiles[t_i][0:u, :],
                            func=mybir.ActivationFunctionType.Tanh,
                            bias=bias_t[0:u, bcol:bcol + 1],
                            scale=1.0,
                        )
                        if ch == NCH - 1:
                            r0 = layer_base[li] + sum(
                                tt["u"] for tt in tiles[:t_i])
                            eng = nc.sync if t_i % 2 == 0 else nc.scalar
                            wr = eng.dma_start(
                                out=accT[hf][r0:r0 + u, :],
                                in_=hs[0:u, :])
                            wl.append(wr)
                return wl

            def emit_out(hf, gxt, ght):
                """Output layer, transposed orientation: gathered block is
                the stationary, weights the moving operand; psum comes out
                [batch, 64] so no output transpose is needed."""
                lay = L[3]
                ystage = fpool.tile([128, HB // 128, 64], f32,
                                    name="ystage", tag=f"ys{hf}")
                for bc in range(HB // 128):
                    pst = pspool.tile([128, 512], f32, name="ps")
                    prev = None
                    for mm in lay["mms"]:
                        gt = gxt if mm.kind == "x" else ght
                        mi = nc.tensor.matmul(
                            out=pst[0:128, 0:64],
                            lhsT=gt[:, mm.blk, bc * 128:(bc + 1) * 128],
                            rhs=wh_t[:, mm.w_off:mm.w_off + mm.w],
                            start=prev is None,
                            stop=mm is lay["mms"][-1],
                            skip_group_check=True,
                        )
                        if prev is not None:
                            tile.add_dep_helper(mi.ins, prev.ins, sync=False)
                        prev = mi
                    nc.vector.tensor_add(
                        out=ystage[:, bc, :],
                        in0=pst[0:128, 0:64],
                        in1=biaso_t[:, 0:64],
                    )
                nc.sync.dma_start(
                    out=y_out[hf * HB:(hf + 1) * HB, :].rearrange(
                        "(c p) o -> p c o", p=128),
                    in_=ystage[:],
                )

            def body(iv=None):
                gx = {}
                gh = {}
                for hf in range(S):
                    for li in range(4):
                        if L[li]["nblk_x"]:
                            gx[(li, hf)] = gxpool.tile(
                                [128, L[li]["nblk_x"], HB], bf16, name="gx",
                                tag=f"gx{li}_{hf}")
                        if L[li]["nblk_h"]:
                            gh[(li, hf)] = ghpool.tile(
                                [128, L[li]["nblk_h"], HB], f8, name="gh",
                                tag=f"gh{li}_{hf}")
                # X gathers upfront, L1 first, halves interleaved
                for li in range(4):
                    for hf in range(S):
                        if (li, hf) in gx:
                            emit_gather("x", li, hf, gx[(li, hf)])
                writes = {0: [], 1: []}
                for li in range(4):
                    for hf in range(S):
                        if li == 3:
                            emit_out(hf, gx.get((3, hf)), gh.get((3, hf)))
                            continue
                        wl = emit_layer(li, hf, gx.get((li, hf)),
                                        gh.get((li, hf)))
                        writes[hf].extend(wl)
                        # H gather for next layer of this half
                        nli = li + 1
                        if (nli, hf) in gh:
                            emit_gather("h", nli, hf, gh[(nli, hf)],
                                        writes=writes[hf])

            if with_loop:
                nit_t = consts.tile([1, 1], mybir.dt.int32)
                nc.sync.dma_start(out=nit_t[:], in_=nit_in[:])
                n = nc.values_load(nit_t[0:1, 0:1], min_val=0, max_val=2048,
                                   skip_runtime_bounds_check=True)
                with tc.For_i(0, n, 1):
                    body()
            else:
                body()

    # Align each gather's SWDGE queue with its Tile-assigned DMASW sem lane
    # (sem->queue is locked 1:1 by the runtime; the scheduler reorders
    # instructions, so emission-order round-robin desyncs).
    from concourse.tile_scheduler import PROC_NAME_TO_IDX

    sw_procs = {PROC_NAME_TO_IDX[f"DMASW{i}"]: i for i in range(8)}

    def _fix_queues(blocks):
        for blk in blocks:
            for inst in blk.instructions:
                if isinstance(inst, mybir.InstDMAGatherAnt):
                    proc = getattr(inst, "bass_scheduled_proc", None)
                    if proc in sw_procs:
                        inst.queue_num = sw_procs[proc] % 4

    _fix_queues(nc.m.functions[0].blocks)
    nc.compile()
    return nc


# ---- runner (same as baseline) ----
class _Runner:
    def __init__(self, nc):
        import jax
        import concourse.mybir as mybir
        from jax.sharding import Mesh, PartitionSpec
        from jax.experimental.shard_map import shard_map
        from concourse.bass2jax import (
            _bass_exec_p, partition_id_tensor, install_neuronx_cc_hook,
        )

        install_neuronx_cc_hook()
        self.jax = jax
        in_names, out_names, out_avals, zero_outs = [], [], [], []
        partition_name = (
            nc.partition_id_tensor.name if nc.partition_id_tensor else None
        )
        for alloc in nc.m.functions[0].allocations:
            if not isinstance(alloc, mybir.MemoryLocationSet):
                continue
            name = alloc.memorylocations[0].name
            if alloc.kind == "ExternalInput":
                if name != partition_name:
                    in_names.append(name)
            elif alloc.kind == "ExternalOutput":
                out_names.append(name)
                shape = tuple(alloc.tensor_shape)
                dtype = mybir.dt.np(alloc.dtype)
                out_avals.append(jax.core.ShapedArray(shape, dtype))
                zero_outs.append(np.zeros(shape, dtype))
        self.n_params = len(in_names)
        self.in_names = in_names[:]
        self.out_names = out_names
        self.out_avals = out_avals
        self.zero_outs = zero_outs
        all_in = in_names + out_names + ([partition_name] if partition_name else [])
        donate = tuple(range(self.n_params, self.n_params + len(out_names)))

        def _body(*args):
            operands = list(args)
            if partition_name is not None:
                operands.append(partition_id_tensor())
            return tuple(
                _bass_exec_p.bind(
                    *operands,
                    out_avals=tuple(out_avals),
                    in_names=tuple(all_in),
                    out_names=tuple(out_names),
                    lowering_input_output_aliases=(),
                    sim_require_finite=True,
                    sim_require_nnan=True,
                    nc=nc,
                )
            )

        devices = jax.devices()[:N_CORES]
        self.mesh = Mesh(np.asarray(devices), ("core",))
        self.sharded = jax.jit(
            shard_map(
                _body, mesh=self.mesh,
                in_specs=(PartitionSpec("core"),) * (self.n_params + len(out_names)),
                out_specs=(PartitionSpec("core"),) * len(out_names),
                check_rep=False,
            ),
            donate_argnums=donate,
            keep_unused=True,
        )

    def prep(self, in_maps, device_put=True):
        per_core = [[np.asarray(m[name]) for name in self.in_names] for m in in_maps]
        arrs = [
            np.concatenate([per_core[c][i] for c in range(N_CORES)], axis=0)
            for i in range(self.n_params)
        ]
        if device_put:
            from jax.sharding import NamedSharding, PartitionSpec

            sh = NamedSharding(self.mesh, PartitionSpec("core"))
            arrs = [self.jax.device_put(a, sh) for a in arrs]
            self.jax.block_until_ready(arrs)
        return arrs

    def run(self, concat_in):
        zeros = [
            np.zeros((N_CORES * z.shape[0], *z.shape[1:]), z.dtype)
            for z in self.zero_outs
        ]
        outs = self.sharded(*concat_in, *zeros)
        self.jax.block_until_ready(outs)
        return outs

    def split(self, out_arrs):
        return [
            {
                name: np.asarray(out_arrs[i]).reshape(
                    N_CORES, *self.out_avals[i].shape
                )[c]
                for i, name in enumerate(self.out_names)
            }
            for c in range(N_CORES)
        ]


def _get(plan_key, plan, with_loop):
    key = (plan_key, with_loop)
    if key not in _cache:
        nc = _build_program(plan, with_loop)
        _cache[key] = _Runner(nc)
    return _cache[key]


def _in_maps(plan, x, niter):
    bf = ml_dtypes.bfloat16
    x = np.asarray(x)
    maps = []
    for c in range(N_CORES):
        xs = x[c * BS:(c + 1) * BS, :]
        m = {
            "xt": np.ascontiguousarray(xs.T).astype(bf),
            "idxw": plan["idx_wrapped"],
            "wh": plan["wh"],
            "biasp": plan["bias"],
            "biaso": np.tile(
                np.asarray(plan["bo_flat"], np.float32).reshape(1, 64),
                (128, 1)),
        }
        if niter is not None:
            m["niter"] = np.array([[niter]], np.int32)
        maps.append(m)
    return maps


def kernel(**inputs):
    niter = inputs.pop("_niter", None)
    x = inputs.pop("x")
    plan = build_plan(**{k: inputs[k] for k in (
        "idx1", "idx2", "idx3", "idxo", "W1", "W2", "W3", "Wo",
        "b1", "b2", "b3", "bo")})
    r = _get("p0", plan, niter is not None)
    ci = r.prep(_in_maps(plan, x, niter), device_put=False)
    outs = r.split(r.run(ci))
    return np.concatenate(
        [outs[c]["y"] for c in range(N_CORES)], axis=0).astype(np.float32)


def bench(inputs, k_hi=129, rounds=8, per=4):
    import time

    inputs = dict(inputs)
    x = inputs.pop("x")
    plan = build_plan(**{k: inputs[k] for k in (
        "idx1", "idx2", "idx3", "idxo", "W1", "W2", "W3", "Wo",
        "b1", "b2", "b3", "bo")})
    r = _get("p0", plan, True)
    ci1 = r.prep(_in_maps(plan, x, 1), device_put=True)
    cih = r.prep(_in_maps(plan, x, k_hi), device_put=True)
    outs = r.split(r.run(ci1))
    y1 = np.concatenate(
        [outs[c]["y"] for c in range(N_CORES)], axis=0).astype(np.float32)
    outs = r.split(r.run(cih))
    yh = np.concatenate(
        [outs[c]["y"] for c in range(N_CORES)], axis=0).astype(np.float32)
    diffs = []
    for _ in range(rounds):
        t1s, ths = [], []
        for _ in range(per):
            t0 = time.perf_counter(); r.run(ci1)
            t1s.append(time.perf_counter() - t0)
            t0 = time.perf_counter(); r.run(cih)
            ths.append(time.perf_counter() - t0)
        diffs.append((min(ths) - min(t1s)) / (k_hi - 1))
    diffs.sort()
    return diffs[len(diffs) // 2], y1, yh
